# revision 23
# baseline (speedup 1.0000x reference)
"""DenseGraphAttentionHead Trainium2 Bass kernel (8-core SPMD row-sharded).

reference math:
    Wh = nodes @ W_w.T + W_b                    [N, 256]
    Wh1 = Wh @ a1_w.T + a1_b                    [N, 1]
    Wh2 = Wh @ a2_w.T + a2_b                    [N, 1]
    scores = leaky_relu(Wh1 + Wh2.T, 0.2)       [N, N]
    attention = softmax(where(edge, scores, -inf), axis=1)
    out = attention @ Wh                        [N, 256]

Key identity: softmax over j is invariant to per-row(i) factors, so with
    p[i] = exp(0.8*Wh1[i]),  q[j] = exp(0.2*Wh2[j]),  r[j] = exp(Wh2[j])
we have  exp(lrelu(Wh1+Wh2) - 0.2*Wh1) = max(q[j], r[j]*p[i])
(branch r*p >= q  <=>  Wh1+Wh2 >= 0, exactly the lrelu branch), hence
    attention_ij ∝ edge_ij * max(q[j], r[j]*p[i]).
The dense exp/lrelu over the 8192x8192 score matrix collapses to one fused
DVE tensor_scalar (mult+max) per 128-chunk plus one tensor_tensor multiply
with the {0,1} edge mask (fp8 in HBM, upcast during the SWDGE DMA); exps
only run on vectors.

Per core c (rows i in [c*1024, (c+1)*1024), scores in [j(part), i(free)]):
  - Wh_aug[j, 0:256] = nodes @ W_w.T (fp16, no bias), col 256 = 1 (rowsum
    column), col 257 = nodes @ v2 = Wh2-c2 (a2 folded into params host-side).
  - X[j, i] = max(q[j], r[j]*p[i]) * mask01[j, i].
  - psum[i, 0:258] += X[:, i_blk].T @ Wh_aug over j chunks; col 256 = softmax
    denominator. out = psum[:, :256]/denom + W_b (softmax rows sum to 1, so
    the +W_b bias commutes with attention@).
"""
import sys
import types

import numpy as np

N_NODES = 8192
IN_DIM = 512
OUT_DIM = 256
ALPHA = 0.2
N_CORES = 8
ROWS = N_NODES // N_CORES          # 1024 rows per core
NCK = N_NODES // 128               # 64 j-chunks of 128
GRP = 4                            # j-chunks per mask-DMA batch

_CACHE = {}


def _ensure_ntff_hook():
    """antenv.axon_hooks is absent in this container; shim it so
    run_bass_kernel_spmd(trace=True) can reach the NTFF profiler."""
    if "antenv.axon_hooks" in sys.modules:
        return
    holder = [None]
    mod = types.ModuleType("antenv.axon_hooks")
    mod.set_axon_ntff_profile_hook = lambda h: holder.__setitem__(0, h)
    mod.get_axon_ntff_profile_hook = lambda: holder[0]
    sys.modules["antenv.axon_hooks"] = mod
    try:
        from trn_agent_boot.trn_boot import _ntff_profile_via_ctypes
        mod.set_axon_ntff_profile_hook(
            _ntff_profile_via_ctypes("/opt/axon/libaxon_pjrt.so"))
    except Exception:
        pass


def _register_gat_op():
    """Register a fused custom DVE op: out = max(in0*s0, s1) * in1.
    One 1x pass replaces the tensor_scalar + tensor_tensor pair and can
    emit fp8 directly. Uses the official per-NEFF custom-DVE table path."""
    from concourse import dve_ops
    from concourse.dve_spec import Spec, Src0, Src1, C0, C1, maxx, lower
    from concourse.dve_spec import _has_src1 as has_src1
    from concourse.dve_uop import DveOpSpec

    name = "GAT_SMAX_MASK"
    if name in dve_ops._SUB_OPCODE_FOR_NAME:
        return next(o for o in dve_ops.OPS if o.name == name)
    spec = Spec(
        body=maxx(Src0 * C0, C1) * Src1,
        reference=lambda in0, in1, s0, s1: np.maximum(in0 * s0, s1) * in1,
    )
    row = dve_ops._CUSTOM_DVE_ROW_BASE + len(dve_ops.OPS)
    shas = {}
    for ver in ("v3", "v4"):
        tmp = DveOpSpec(name=name, opcode=row, uops=lower(spec, ver=ver),
                        rd1_en=has_src1(spec))
        shas[ver] = tmp.sha(ver)
    op = dve_ops.DveOp(name, spec, subdim=False, uops_sha=shas)
    dve_ops.OPS.append(op)
    dve_ops._SUB_OPCODE_FOR_NAME[name] = row
    return op


# chunks >= FP8_START go through fp8 X + fp8 wh_aug with DoubleRow matmuls
FP8_START = 48

# X-production strategy per (half, group); default "dve2" (DVE ts+tt).
# "pool"/"act" offload part of each group to the Pool/Act engines to keep
# the DVE off the critical path; *8 variants emit fp8 X for DoubleRow.
X_VARIANTS = {
    (0, 12): "pool8", (0, 13): "pool8", (1, 12): "pool8", (1, 13): "pool8",
    (0, 14): "act8", (0, 15): "act8", (1, 14): "act8", (1, 15): "act8",
    (1, 0): "pool", (1, 1): "pool", (0, 10): "pool", (0, 11): "pool",
    (1, 10): "pool", (1, 11): "pool",
    (0, 8): "act", (0, 9): "act", (1, 2): "act", (1, 3): "act",
    (1, 8): "act", (1, 9): "act",
}


def _build_nc():
    import concourse.bacc as bacc
    import concourse.tile as tile
    from concourse import mybir

    gat_op = _register_gat_op()

    F16 = mybir.dt.float16
    BF16 = mybir.dt.bfloat16
    F32 = mybir.dt.float32
    FP8 = mybir.dt.float8e4
    ADD = mybir.AluOpType.add
    MULT = mybir.AluOpType.mult
    MAX = mybir.AluOpType.max
    EXP = mybir.ActivationFunctionType.Exp
    RELU = mybir.ActivationFunctionType.Relu
    DR = mybir.MatmulPerfMode.DoubleRow

    nc = bacc.Bacc("TRN2", target_bir_lowering=False, debug=False,
                   num_devices=N_CORES)

    nodesT_d = nc.dram_tensor("nodesT", [IN_DIM, N_NODES], F16,
                              kind="ExternalInput")
    maskm_d = nc.dram_tensor("maskm", [N_NODES, ROWS], mybir.dt.float8e4,
                             kind="ExternalInput")
    # cols 0:256 = W_w.T, col 256 = v2 (a2 folded), col 257 = v1 (a1 folded)
    wtaug_d = nc.dram_tensor("wt_aug", [IN_DIM, 258], F16,
                             kind="ExternalInput")
    wb_d = nc.dram_tensor("wb_bc", [128, OUT_DIM], F32, kind="ExternalInput")
    # col 0 = 0.2*c2 (q bias), col 1 = 0.8*c2 (rt bias), col 2 = c1
    c2_d = nc.dram_tensor("c2qr", [128, 3], F32, kind="ExternalInput")
    out_d = nc.dram_tensor("out", [ROWS, OUT_DIM], F32, kind="ExternalOutput")

    with tile.TileContext(nc) as tc:
        with (
            tc.tile_pool(name="consts", bufs=1) as consts,
            tc.tile_pool(name="ndpool", bufs=5) as ndpool,
            tc.tile_pool(name="grpp", bufs=4) as grpp,
            tc.tile_pool(name="outp", bufs=2) as outp,
        ):
            # ---- constants. Critical path: ndT0 + wt gate pw1 -> p_row
            # and the Wh build. One batched DMA per tensor, spread over
            # queues so nothing serializes behind bulk traffic.
            ndT0 = ndpool.tile([128, 4, 1024], F16, name="ndT", tag="ndT")
            for h2 in range(2):
                nd_src = nodesT_d[:, h2 * 512:(h2 + 1) * 512]
                nd_src = nd_src.rearrange("(d p) i -> p d i", p=128)
                nc.sync.dma_start(ndT0[:, :, h2 * 512:(h2 + 1) * 512], nd_src)

            wt = consts.tile([128, 4, 258], F16, name="wt", tag="wt")
            nc.scalar.dma_start(
                wt[:], wtaug_d.rearrange("(d p) f -> p d f", p=128))
            wt_t = [wt[:, d4, 0:257] for d4 in range(4)]
            v1_t = [wt[:, d4, 257:258] for d4 in range(4)]

            c2qr = consts.tile([128, 3], F32)
            nc.gpsimd.dma_start(c2qr[:], c2_d[:])
            c1 = c2qr[0:1, 2:3]
            wb_bc = consts.tile([128, OUT_DIM], F32)
            nc.gpsimd.dma_start(wb_bc[:], wb_d[:])

            # wh_aug chunk layout: cols 0:256 = Wh (bf16), col 256 = 1.0
            # (softmax-denominator column, memset once below; the per-chunk
            # copies never touch it). Wh2 never lands in SBUF wh_aug; it is
            # extracted straight from PSUM col 256 of each chunk matmul.
            wh_aug = consts.tile([128, NCK, 257], BF16)
            nc.gpsimd.memset(wh_aug[:, :, 256:257], 1.0)
            # fp8 copy of chunks FP8_START.. for DoubleRow; 272-byte chunk
            # pitch keeps the 3D moving-AP step 16B-aligned
            wh_aug8 = consts.tile([128, NCK - FP8_START, 272], FP8)
            nc.gpsimd.memset(wh_aug8[:, :, 256:257], 1.0)
            wh2f32 = consts.tile([128, NCK], F32)
            q128 = consts.tile([128, NCK], F32)
            nq128 = consts.tile([128, NCK], F32)
            r128 = consts.tile([128, NCK], F32)

            HALF = 512
            NG = NCK // GRP            # 16 groups of GRP chunks per half
            import contextlib
            psA_stack = contextlib.ExitStack()
            with tc.tile_pool(name="psB", bufs=1, space="PSUM") as psB:
                psA = psA_stack.enter_context(
                    tc.tile_pool(name="psA", bufs=2, space="PSUM"))
                # ---- Wh1 row for own block + p = exp(0.8*Wh1) broadcast ----
                # pw1 borrows a pwh2-pool buffer (PSUM budget: 2x2 pwh2
                # banks + 4 acc banks = 8)
                wh1row = consts.tile([1, ROWS], F16)
                for h2 in range(2):
                    pw1t = psA.tile([128, 2, 512], F32, name="pwh2",
                                    tag="pwh2", bufs=2)
                    pw1 = pw1t[0:1, 0, :]
                    for d4 in range(4):
                        nc.tensor.matmul(
                            pw1, v1_t[d4],
                            ndT0[:, d4, h2 * 512:(h2 + 1) * 512],
                            start=(d4 == 0), stop=(d4 == 3),
                            skip_group_check=True)
                    nc.vector.tensor_scalar(
                        wh1row[:, h2 * 512:(h2 + 1) * 512], pw1, c1[:],
                        None, op0=ADD)
                p_row = consts.tile([1, ROWS], BF16)
                nc.scalar.activation(p_row[:], wh1row[:], EXP, scale=ALPHA * 4)

                p_b = consts.tile([128, ROWS], BF16)

                def build_wh_block(b, ndT=None):
                    if ndT is None:
                        ndT = ndpool.tile([128, 4, 1024], F16, name="ndT",
                                          tag="ndT")
                        # blk1 on scalar so it does not serialize behind
                        # later blocks on sync
                        dma_eng = nc.scalar if b == 1 else nc.sync
                        nd_src = nodesT_d[:, b * 1024:(b + 1) * 1024]
                        nd_src = nd_src.rearrange("(d p) i -> p d i", p=128)
                        dma_eng.dma_start(ndT[:], nd_src)
                    for cp in range(4):
                        ck0 = b * 8 + cp * 2
                        # chunk pair in one 2-bank PSUM tile -> batched
                        # (2-chunk) extracts and copies halve ACT overhead
                        pwh2 = psA.tile([128, 2, 512], F32, name="pwh2",
                                        tag="pwh2", bufs=2)
                        for k2 in range(2):
                            ckl = cp * 2 + k2
                            for d4 in range(4):
                                nc.tensor.matmul(
                                    pwh2[:, k2, 0:257],
                                    ndT[:, d4, ckl * 128:(ckl + 1) * 128],
                                    wt_t[d4],
                                    start=(d4 == 0), stop=(d4 == 3),
                                    skip_group_check=True)
                        # wh2 extract first (feeds q/r -> X pipeline); DVE
                        # for the first blocks where it is idle, ACT after
                        if b < 2:
                            nc.vector.tensor_copy(wh2f32[:, ck0:ck0 + 2],
                                                  pwh2[:, :, 256:257])
                        else:
                            nc.scalar.copy(wh2f32[:, ck0:ck0 + 2],
                                           pwh2[:, :, 256:257])
                        if cp == 3:
                            sl = slice(b * 8, (b + 1) * 8)
                            nc.scalar.activation(q128[:, sl], wh2f32[:, sl],
                                                 EXP, scale=ALPHA,
                                                 bias=c2qr[:, 0:1])
                            nc.scalar.activation(r128[:, sl], wh2f32[:, sl],
                                                 EXP, scale=1.0,
                                                 bias=c2qr[:, 1:2])
                            nc.scalar.mul(nq128[:, sl], q128[:, sl], -1.0)

                        nc.scalar.copy(wh_aug[:, ck0:ck0 + 2, 0:256],
                                       pwh2[:, :, 0:256])
                        if ck0 >= FP8_START:
                            c8 = ck0 - FP8_START
                            nc.scalar.copy(wh_aug8[:, c8:c8 + 2, 0:256],
                                           pwh2[:, :, 0:256])

                build_wh_block(0, ndT=ndT0)
                build_wh_block(1)

                # ---- main sweep over i-halves, Wh blocks interleaved in
                # PE program order during the first half ----
                def emit_mask(h, g, suffix, bufs, dt=BF16):
                    mgrp = grpp.tile([128, GRP, HALF], dt,
                                     name=f"mgrp{suffix}",
                                     tag=f"mgrp{suffix}", bufs=bufs)
                    msrc = maskm_d[g * GRP * 128:(g + 1) * GRP * 128,
                                   h * HALF:(h + 1) * HALF]
                    msrc = msrc.rearrange("(c p) i -> p c i", p=128)
                    nc.gpsimd.dma_start(mgrp[:], msrc)  # fp8->bf16 cast
                    return mgrp

                def emit_x(h, g, suffix, bufs, xbufs=None, mgrp=None):
                    variant = X_VARIANTS.get((h, g), "dve2")
                    is8 = variant.endswith("8")
                    base = variant[:-1] if is8 else variant
                    if mgrp is None:
                        mgrp = emit_mask(h, g, suffix, bufs)
                    xtag = f"xgrp{suffix}" + ("8" if is8 else "")
                    xgrp = grpp.tile([128, GRP, HALF], FP8 if is8 else BF16,
                                     name=xtag, tag=xtag,
                                     bufs=xbufs or 5)
                    pb_h = p_b[:, h * HALF:(h + 1) * HALF]
                    if base == "fused":
                        for ckl in range(GRP):
                            ck = g * GRP + ckl
                            nc.vector._custom_dve(
                                gat_op, out=xgrp[:, ckl, :], in0=pb_h,
                                in1=mgrp[:, ckl, :],
                                s0=r128[:, ck:ck + 1], s1=q128[:, ck:ck + 1])
                        return xgrp
                    if base == "act":
                        agrp = grpp.tile([128, GRP, HALF], BF16,
                                         name=f"agrp{suffix}",
                                         tag=f"agrp{suffix}", bufs=3)
                        for ckl in range(GRP):
                            ck = g * GRP + ckl
                            nc.scalar.activation(
                                agrp[:, ckl, :], pb_h, RELU,
                                scale=r128[:, ck:ck + 1],
                                bias=nq128[:, ck:ck + 1])
                            nc.vector.scalar_tensor_tensor(
                                xgrp[:, ckl, :], agrp[:, ckl, :],
                                q128[:, ck:ck + 1], mgrp[:, ckl, :],
                                op0=ADD, op1=MULT)
                        return xgrp
                    sgrp = grpp.tile([128, GRP, HALF], BF16,
                                     name=f"sgrp{suffix}",
                                     tag=f"sgrp{suffix}", bufs=min(bufs, 4))
                    for ckl in range(GRP):
                        ck = g * GRP + ckl
                        nc.vector.tensor_scalar(
                            sgrp[:, ckl, :], pb_h,
                            r128[:, ck:ck + 1], q128[:, ck:ck + 1],
                            op0=MULT, op1=MAX)
                    if base == "pool":
                        nc.gpsimd.tensor_tensor(xgrp[:], sgrp[:], mgrp[:],
                                                op=MULT)
                    else:
                        nc.vector.tensor_tensor(xgrp[:], sgrp[:], mgrp[:],
                                                op=MULT)
                    return xgrp

                pre_masks = [emit_mask(0, 0, "", 6),
                             emit_mask(0, 1, "", 6),
                             emit_mask(0, 2, "", 6)]
                nc.gpsimd.partition_broadcast(p_b[:], p_row[:])

                def drain_one(h, ib, accs, eng):
                    recip = outp.tile([128, 1], F32, name="recip",
                                      tag="recip", bufs=4)
                    nc.vector.reciprocal(recip[:], accs[ib][:, 256:257])
                    o = outp.tile([128, OUT_DIM], F32, name="o", tag="o",
                                  bufs=4)
                    nc.vector.scalar_tensor_tensor(
                        o[:], accs[ib][:, 0:OUT_DIM], recip[:], wb_bc[:],
                        op0=MULT, op1=ADD)
                    r0 = h * HALF + ib * 128
                    eng.dma_start(out_d[r0:r0 + 128, :], o[:])

                def group_mms(g, ib, xgrp, acc, is8):
                    if is8:
                        # DoubleRow: one fp8 matmul per chunk pair, 256-deep
                        # contraction ([128, 2, ...] APs)
                        for k2 in range(2):
                            ck = g * GRP + k2 * 2
                            c8 = ck - FP8_START
                            nc.tensor.matmul(
                                acc[:],
                                xgrp[:, k2 * 2:k2 * 2 + 2,
                                     ib * 128:(ib + 1) * 128],
                                wh_aug8[:, c8:c8 + 2, 0:257],
                                start=False, stop=(ck == NCK - 2),
                                perf_mode=DR, skip_group_check=True)
                    else:
                        for ckl in range(GRP):
                            ck = g * GRP + ckl
                            nc.tensor.matmul(
                                acc[:],
                                xgrp[:, ckl, ib * 128:(ib + 1) * 128],
                                wh_aug[:, ck, 0:257],
                                start=(ck == 0), stop=(ck == NCK - 1),
                                skip_group_check=True)

                def accumulate(h, accs, prefetched, pre_masks,
                               drain_last=False):
                    for g in range(NG):
                        if h == 0 and g < NCK // 8 - 2:
                            build_wh_block(g + 2)
                        if h == 1 and g < len(prefetched):
                            xgrp = prefetched[g]
                        elif h == 0 and g < len(pre_masks):
                            xgrp = emit_x(h, g, "", 6, mgrp=pre_masks[g])
                        else:
                            xgrp = emit_x(h, g, "", 6)
                        is8 = X_VARIANTS.get((h, g), "").endswith("8")
                        if drain_last and g == NG - 1:
                            # ib-major for the final group: each acc bank
                            # finishes early and its drain overlaps the
                            # remaining matmuls
                            engs = [nc.sync, nc.scalar, nc.gpsimd, nc.sync]
                            for ib in range(4):
                                group_mms(g, ib, xgrp, accs[ib], is8)
                                drain_one(h, ib, accs, engs[ib])
                        elif is8:
                            for ib in range(4):
                                group_mms(g, ib, xgrp, accs[ib], is8)
                        else:
                            for ckl in range(GRP):
                                ck = g * GRP + ckl
                                for ib in range(4):
                                    nc.tensor.matmul(
                                        accs[ib][:],
                                        xgrp[:, ckl, ib * 128:(ib + 1) * 128],
                                        wh_aug[:, ck, 0:257],
                                        start=(ck == 0), stop=(ck == NCK - 1),
                                        skip_group_check=True)

                def drain(h, accs):
                    engs = [nc.sync, nc.scalar, nc.sync, nc.scalar]
                    for ib in range(4):
                        drain_one(h, ib, accs, engs[ib])

                accs0 = [psB.tile([128, 257], F32, name=f"acc{ib}",
                                  tag=f"acc{ib}") for ib in range(4)]
                accumulate(0, accs0, [], pre_masks)
                # build h=1's first score groups while the DVE is idle at
                # the tail of h=0
                prefetched = [emit_x(1, 0, "", 6, xbufs=5),
                              emit_x(1, 1, "", 6, xbufs=5),
                              emit_x(1, 2, "", 6, xbufs=5),
                              emit_x(1, 3, "", 6, xbufs=5)]
                psA_stack.close()   # free pwh/pw1 PSUM banks for h1 accs
                with tc.tile_pool(name="psC", bufs=1, space="PSUM") as psC:
                    accs1 = [psC.tile([128, 257], F32, name=f"acc1{ib}",
                                      tag=f"acc1{ib}") for ib in range(4)]
                    drain(0, accs0)
                    accumulate(1, accs1, prefetched, pre_masks,
                               drain_last=True)
    nc.compile()
    return nc


def _get_nc():
    if "nc" not in _CACHE:
        _CACHE["nc"] = _build_nc()
    return _CACHE["nc"]


def _prep_in_maps(nodes, edge_mat, W_w, W_b, a1_w, a1_b, a2_w, a2_b):
    f16 = np.float16
    nodes = np.asarray(nodes, dtype=np.float32)
    edge_mat = np.asarray(edge_mat, dtype=bool)
    W_w = np.asarray(W_w, dtype=np.float32)
    W_b = np.asarray(W_b, dtype=np.float32)
    a1_w = np.asarray(a1_w, dtype=np.float32)
    a1_b = np.asarray(a1_b, dtype=np.float32)
    a2_w = np.asarray(a2_w, dtype=np.float32)
    a2_b = np.asarray(a2_b, dtype=np.float32)

    nodesT = np.ascontiguousarray(nodes.T).astype(f16)          # [512, 8192]
    v1 = (W_w.T @ a1_w[0]).astype(f16)[:, None]                 # [512, 1]
    v2 = (W_w.T @ a2_w[0]).astype(f16)[:, None]
    wt_aug = np.concatenate([W_w.T.astype(f16), v2, v1], axis=1)  # [512, 258]
    c1v = float(W_b @ a1_w[0]) + float(a1_b[0])
    c2v = float(W_b @ a2_w[0]) + float(a2_b[0])
    c2qr = np.broadcast_to(
        np.array([ALPHA * c2v, c2v, c1v], np.float32)[None, :],
        (128, 3)).copy()
    wb_bc = np.ascontiguousarray(
        np.broadcast_to(W_b[None, :], (128, OUT_DIM))).astype(np.float32)
    # multiplicative {0,1} mask, transposed, fp8 (cast to fp16 during DMA)
    import ml_dtypes
    maskT = np.where(edge_mat, 1, 0).astype(ml_dtypes.float8_e4m3fn).T

    in_maps = []
    for c in range(N_CORES):
        sl = slice(c * ROWS, (c + 1) * ROWS)
        # permute node columns so this core's own block comes first; the
        # j-order of Wh/X/mask rows follows the same permutation (softmax
        # sums over j are permutation-invariant, output rows stay i-ordered)
        nT = np.concatenate(
            [nodesT[:, sl], nodesT[:, :sl.start], nodesT[:, sl.stop:]],
            axis=1)
        mcol = maskT[:, sl]
        mperm = np.concatenate(
            [mcol[sl.start:sl.stop], mcol[:sl.start], mcol[sl.stop:]], axis=0)
        in_maps.append({
            "nodesT": np.ascontiguousarray(nT),
            "maskm": np.ascontiguousarray(mperm),
            "wt_aug": wt_aug,
            "wb_bc": wb_bc,
            "c2qr": c2qr,
        })
    return in_maps


def _run(inputs, trace=False, trace_cores=None):
    from concourse.bass_utils import run_bass_kernel_spmd
    if trace:
        _ensure_ntff_hook()
    nc = _get_nc()
    in_maps = _prep_in_maps(**inputs)
    res = run_bass_kernel_spmd(nc, in_maps, list(range(N_CORES)),
                               trace=trace, trace_cores=trace_cores)
    out = np.concatenate([res.results[c]["out"] for c in range(N_CORES)],
                         axis=0)
    return out, res


def kernel(**inputs) -> np.ndarray:
    out, _ = _run(inputs, trace=False)
    return out


# revision 30
# speedup vs baseline: 1.1815x; 1.1815x over previous
"""DenseGraphAttentionHead Trainium2 Bass kernel (8-core SPMD row-sharded).

reference math:
    Wh = nodes @ W_w.T + W_b                    [N, 256]
    Wh1 = Wh @ a1_w.T + a1_b                    [N, 1]
    Wh2 = Wh @ a2_w.T + a2_b                    [N, 1]
    scores = leaky_relu(Wh1 + Wh2.T, 0.2)       [N, N]
    attention = softmax(where(edge, scores, -inf), axis=1)
    out = attention @ Wh                        [N, 256]

Key identity: softmax over j is invariant to per-row(i) factors, so with
    p[i] = exp(0.8*Wh1[i]),  q[j] = exp(0.2*Wh2[j]),  r[j] = exp(Wh2[j])
we have  exp(lrelu(Wh1+Wh2) - 0.2*Wh1) = max(q[j], r[j]*p[i])
(branch r*p >= q  <=>  Wh1+Wh2 >= 0, exactly the lrelu branch), hence
    attention_ij ∝ edge_ij * max(q[j], r[j]*p[i]).
The dense exp/lrelu over the 8192x8192 score matrix collapses to one fused
DVE tensor_scalar (mult+max) per 128-chunk plus one tensor_tensor multiply
with the {0,1} edge mask (fp8 in HBM, upcast during the SWDGE DMA); exps
only run on vectors.

Per core c (rows i in [c*1024, (c+1)*1024), scores in [j(part), i(free)]):
  - Wh_aug[j, 0:256] = nodes @ W_w.T (fp16, no bias), col 256 = 1 (rowsum
    column), col 257 = nodes @ v2 = Wh2-c2 (a2 folded into params host-side).
  - X[j, i] = max(q[j], r[j]*p[i]) * mask01[j, i].
  - psum[i, 0:258] += X[:, i_blk].T @ Wh_aug over j chunks; col 256 = softmax
    denominator. out = psum[:, :256]/denom + W_b (softmax rows sum to 1, so
    the +W_b bias commutes with attention@).
"""
import sys
import types

import numpy as np

N_NODES = 8192
IN_DIM = 512
OUT_DIM = 256
ALPHA = 0.2
N_CORES = 8
ROWS = N_NODES // N_CORES          # 1024 rows per core
NCK = N_NODES // 128               # 64 j-chunks of 128
GRP = 4                            # j-chunks per mask-DMA batch

_CACHE = {}


def _ensure_ntff_hook():
    """antenv.axon_hooks is absent in this container; shim it so
    run_bass_kernel_spmd(trace=True) can reach the NTFF profiler."""
    if "antenv.axon_hooks" in sys.modules:
        return
    holder = [None]
    mod = types.ModuleType("antenv.axon_hooks")
    mod.set_axon_ntff_profile_hook = lambda h: holder.__setitem__(0, h)
    mod.get_axon_ntff_profile_hook = lambda: holder[0]
    sys.modules["antenv.axon_hooks"] = mod
    try:
        from trn_agent_boot.trn_boot import _ntff_profile_via_ctypes
        mod.set_axon_ntff_profile_hook(
            _ntff_profile_via_ctypes("/opt/axon/libaxon_pjrt.so"))
    except Exception:
        pass


def _register_gat_op():
    """Register a fused custom DVE op: out = max(in0*s0, s1) * in1.
    One 1x pass replaces the tensor_scalar + tensor_tensor pair and can
    emit fp8 directly. Uses the official per-NEFF custom-DVE table path."""
    from concourse import dve_ops
    from concourse.dve_spec import Spec, Src0, Src1, C0, C1, maxx, lower
    from concourse.dve_spec import _has_src1 as has_src1
    from concourse.dve_uop import DveOpSpec

    name = "GAT_SMAX_MASK"
    if name in dve_ops._SUB_OPCODE_FOR_NAME:
        return next(o for o in dve_ops.OPS if o.name == name)
    spec = Spec(
        body=maxx(Src0 * C0, C1) * Src1,
        reference=lambda in0, in1, s0, s1: np.maximum(in0 * s0, s1) * in1,
    )
    row = dve_ops._CUSTOM_DVE_ROW_BASE + len(dve_ops.OPS)
    shas = {}
    for ver in ("v3", "v4"):
        tmp = DveOpSpec(name=name, opcode=row, uops=lower(spec, ver=ver),
                        rd1_en=has_src1(spec))
        shas[ver] = tmp.sha(ver)
    op = dve_ops.DveOp(name, spec, subdim=False, uops_sha=shas)
    dve_ops.OPS.append(op)
    dve_ops._SUB_OPCODE_FOR_NAME[name] = row
    return op


# X-production strategy per (half, group); default "dve2" (DVE ts+tt).
# "act" offloads the score op of a group to the Act engine (relu identity:
# max(q, r*p) = relu(r*p - q) + q) so the DVE only runs one combine pass.
# Measured: Pool-engine offload and fp8 DoubleRow are both net losses
# (SBUF/queue contention; DR spacing 214ns vs 2x112ns bf16).
X_VARIANTS = {(0, g): "act" for g in range(8, 14)}
X_VARIANTS.update({(1, g): "act" for g in range(0, 6)})


def _build_nc():
    import concourse.bacc as bacc
    import concourse.tile as tile
    from concourse import mybir

    gat_op = _register_gat_op()

    F16 = mybir.dt.float16
    BF16 = mybir.dt.bfloat16
    F32 = mybir.dt.float32
    FP8 = mybir.dt.float8e4
    ADD = mybir.AluOpType.add
    MULT = mybir.AluOpType.mult
    MAX = mybir.AluOpType.max
    EXP = mybir.ActivationFunctionType.Exp
    RELU = mybir.ActivationFunctionType.Relu
    DR = mybir.MatmulPerfMode.DoubleRow

    nc = bacc.Bacc("TRN2", target_bir_lowering=False, debug=False,
                   num_devices=N_CORES)

    nodesT_d = nc.dram_tensor("nodesT", [IN_DIM, N_NODES], F16,
                              kind="ExternalInput")
    maskm_d = nc.dram_tensor("maskm", [N_NODES, ROWS], mybir.dt.float8e4,
                             kind="ExternalInput")
    # cols 0:256 = W_w.T, col 256 = v2 (a2 folded), col 257 = v1 (a1 folded)
    wtaug_d = nc.dram_tensor("wt_aug", [IN_DIM, 258], F16,
                             kind="ExternalInput")
    wb_d = nc.dram_tensor("wb_bc", [128, OUT_DIM], F32, kind="ExternalInput")
    # col 0 = 0.2*c2 (q bias), col 1 = 0.8*c2 (rt bias), col 2 = c1
    c2_d = nc.dram_tensor("c2qr", [128, 3], F32, kind="ExternalInput")
    out_d = nc.dram_tensor("out", [ROWS, OUT_DIM], F32, kind="ExternalOutput")

    with tile.TileContext(nc) as tc:
        with (
            tc.tile_pool(name="consts", bufs=1) as consts,
            tc.tile_pool(name="ndpool", bufs=5) as ndpool,
            tc.tile_pool(name="grpp", bufs=4) as grpp,
            tc.tile_pool(name="outp", bufs=2) as outp,
        ):
            # ---- constants. Critical path: ndT0 + wt gate pw1 -> p_row
            # and the Wh build. One batched DMA per tensor, spread over
            # queues so nothing serializes behind bulk traffic.
            # block 0 as two half tiles: pw1/h0 and build chunks 0-3 start
            # as soon as the first 512 columns land
            ndT0h = []
            for h2 in range(2):
                t = ndpool.tile([128, 4, 512], F16, name="ndT0", tag="ndT0",
                                bufs=2)
                nd_src = nodesT_d[:, h2 * 512:(h2 + 1) * 512]
                nd_src = nd_src.rearrange("(d p) i -> p d i", p=128)
                nc.sync.dma_start(t[:], nd_src)
                ndT0h.append(t)

            wt = consts.tile([128, 4, 258], F16, name="wt", tag="wt")
            nc.scalar.dma_start(
                wt[:], wtaug_d.rearrange("(d p) f -> p d f", p=128))
            wt_t = [wt[:, d4, 0:257] for d4 in range(4)]
            v1_t = [wt[:, d4, 257:258] for d4 in range(4)]

            c2qr = consts.tile([128, 3], F32)
            nc.gpsimd.dma_start(c2qr[:], c2_d[:])
            c1 = c2qr[0:1, 2:3]
            wb_bc = consts.tile([128, OUT_DIM], F32)
            nc.gpsimd.dma_start(wb_bc[:], wb_d[:])

            # wh_aug chunk layout: cols 0:256 = Wh (bf16), col 256 = 1.0
            # (softmax-denominator column, memset once below; the per-chunk
            # copies never touch it). Wh2 never lands in SBUF wh_aug; it is
            # extracted straight from PSUM col 256 of each chunk matmul.
            wh_aug = consts.tile([128, NCK, 257], BF16)
            nc.gpsimd.memset(wh_aug[:, :, 256:257], 1.0)
            wh2f32 = consts.tile([128, NCK], F32)
            q128 = consts.tile([128, NCK], F32)
            nq128 = consts.tile([128, NCK], F32)
            r128 = consts.tile([128, NCK], F32)

            HALF = 512
            NG = NCK // GRP            # 16 groups of GRP chunks per half
            import contextlib
            psA_stack = contextlib.ExitStack()
            with tc.tile_pool(name="psB", bufs=1, space="PSUM") as psB:
                psA = psA_stack.enter_context(
                    tc.tile_pool(name="psA", bufs=2, space="PSUM"))
                # ---- Wh1 row for own block + p = exp(0.8*Wh1) broadcast ----
                # pw1 borrows a pwh2-pool buffer (PSUM budget: 2x2 pwh2
                # banks + 4 acc banks = 8)
                wh1row = consts.tile([1, ROWS], F16)
                for h2 in range(2):
                    pw1t = psA.tile([128, 2, 512], F32, name="pwh2",
                                    tag="pwh2", bufs=2)
                    pw1 = pw1t[0:1, 0, :]
                    for d4 in range(4):
                        nc.tensor.matmul(
                            pw1, v1_t[d4], ndT0h[h2][:, d4, :],
                            start=(d4 == 0), stop=(d4 == 3),
                            skip_group_check=True)
                    nc.vector.tensor_scalar(
                        wh1row[:, h2 * 512:(h2 + 1) * 512], pw1, c1[:],
                        None, op0=ADD)
                p_row = consts.tile([1, ROWS], BF16)
                nc.scalar.activation(p_row[:], wh1row[:], EXP, scale=ALPHA * 4)

                p_b = consts.tile([128, ROWS], BF16)

                def build_wh_block(b, ndT=None):
                    if b != 0 and ndT is None:
                        ndT = ndpool.tile([128, 4, 1024], F16, name="ndT",
                                          tag="ndT")
                        # blk1 on scalar so it does not serialize behind
                        # later blocks on sync
                        dma_eng = nc.scalar if b == 1 else nc.sync
                        nd_src = nodesT_d[:, b * 1024:(b + 1) * 1024]
                        nd_src = nd_src.rearrange("(d p) i -> p d i", p=128)
                        dma_eng.dma_start(ndT[:], nd_src)

                    def nd_ap(ckl, d4):
                        if b == 0:
                            h2, c4 = ckl // 4, ckl % 4
                            return ndT0h[h2][:, d4, c4 * 128:(c4 + 1) * 128]
                        return ndT[:, d4, ckl * 128:(ckl + 1) * 128]

                    for cp in range(4):
                        ck0 = b * 8 + cp * 2
                        # chunk pair in one 2-bank PSUM tile -> batched
                        # (2-chunk) extracts and copies halve ACT overhead
                        pwh2 = psA.tile([128, 2, 512], F32, name="pwh2",
                                        tag="pwh2", bufs=2)
                        for k2 in range(2):
                            ckl = cp * 2 + k2
                            for d4 in range(4):
                                nc.tensor.matmul(
                                    pwh2[:, k2, 0:257], nd_ap(ckl, d4),
                                    wt_t[d4],
                                    start=(d4 == 0), stop=(d4 == 3),
                                    skip_group_check=True)
                        # wh2 extract first (feeds q/r -> X pipeline); DVE
                        # for the first blocks where it is idle, ACT after
                        if b < 2:
                            nc.vector.tensor_copy(wh2f32[:, ck0:ck0 + 2],
                                                  pwh2[:, :, 256:257])
                        else:
                            nc.scalar.copy(wh2f32[:, ck0:ck0 + 2],
                                           pwh2[:, :, 256:257])
                        if cp == 3:
                            sl = slice(b * 8, (b + 1) * 8)
                            nc.scalar.activation(q128[:, sl], wh2f32[:, sl],
                                                 EXP, scale=ALPHA,
                                                 bias=c2qr[:, 0:1])
                            nc.scalar.activation(r128[:, sl], wh2f32[:, sl],
                                                 EXP, scale=1.0,
                                                 bias=c2qr[:, 1:2])
                            nc.scalar.mul(nq128[:, sl], q128[:, sl], -1.0)

                        nc.scalar.copy(wh_aug[:, ck0:ck0 + 2, 0:256],
                                       pwh2[:, :, 0:256])

                build_wh_block(0)
                build_wh_block(1)

                # ---- main sweep over i-halves, Wh blocks interleaved in
                # PE program order during the first half ----
                def emit_mask(h, g, suffix, bufs, dt=BF16):
                    mgrp = grpp.tile([128, GRP, HALF], dt,
                                     name=f"mgrp{suffix}",
                                     tag=f"mgrp{suffix}", bufs=bufs)
                    msrc = maskm_d[g * GRP * 128:(g + 1) * GRP * 128,
                                   h * HALF:(h + 1) * HALF]
                    msrc = msrc.rearrange("(c p) i -> p c i", p=128)
                    nc.gpsimd.dma_start(mgrp[:], msrc)  # fp8->bf16 cast
                    return mgrp

                def emit_x(h, g, suffix, bufs, xbufs=None, mgrp=None):
                    variant = X_VARIANTS.get((h, g), "dve2")
                    is8 = variant.endswith("8")
                    base = variant[:-1] if is8 else variant
                    if mgrp is None:
                        mgrp = emit_mask(h, g, suffix, bufs)
                    xtag = f"xgrp{suffix}" + ("8" if is8 else "")
                    xgrp = grpp.tile([128, GRP, HALF], FP8 if is8 else BF16,
                                     name=xtag, tag=xtag,
                                     bufs=xbufs or 5)
                    pb_h = p_b[:, h * HALF:(h + 1) * HALF]
                    if base == "fused":
                        for ckl in range(GRP):
                            ck = g * GRP + ckl
                            nc.vector._custom_dve(
                                gat_op, out=xgrp[:, ckl, :], in0=pb_h,
                                in1=mgrp[:, ckl, :],
                                s0=r128[:, ck:ck + 1], s1=q128[:, ck:ck + 1])
                        return xgrp
                    if base == "act":
                        agrp = grpp.tile([128, GRP, HALF], BF16,
                                         name=f"agrp{suffix}",
                                         tag=f"agrp{suffix}", bufs=3)
                        for ckl in range(GRP):
                            ck = g * GRP + ckl
                            nc.scalar.activation(
                                agrp[:, ckl, :], pb_h, RELU,
                                scale=r128[:, ck:ck + 1],
                                bias=nq128[:, ck:ck + 1])
                            nc.vector.scalar_tensor_tensor(
                                xgrp[:, ckl, :], agrp[:, ckl, :],
                                q128[:, ck:ck + 1], mgrp[:, ckl, :],
                                op0=ADD, op1=MULT)
                        return xgrp
                    sgrp = grpp.tile([128, GRP, HALF], BF16,
                                     name=f"sgrp{suffix}",
                                     tag=f"sgrp{suffix}", bufs=min(bufs, 4))
                    for ckl in range(GRP):
                        ck = g * GRP + ckl
                        nc.vector.tensor_scalar(
                            sgrp[:, ckl, :], pb_h,
                            r128[:, ck:ck + 1], q128[:, ck:ck + 1],
                            op0=MULT, op1=MAX)
                    if base == "pool":
                        nc.gpsimd.tensor_tensor(xgrp[:], sgrp[:], mgrp[:],
                                                op=MULT)
                    else:
                        nc.vector.tensor_tensor(xgrp[:], sgrp[:], mgrp[:],
                                                op=MULT)
                    return xgrp

                pre_masks = [emit_mask(0, 0, "", 6),
                             emit_mask(0, 1, "", 6),
                             emit_mask(0, 2, "", 6)]
                nc.gpsimd.partition_broadcast(p_b[:], p_row[:])

                def drain_one(h, ib, accs, eng):
                    recip = outp.tile([128, 1], F32, name="recip",
                                      tag="recip", bufs=4)
                    nc.vector.reciprocal(recip[:], accs[ib][:, 256:257])
                    o = outp.tile([128, OUT_DIM], F32, name="o", tag="o",
                                  bufs=4)
                    nc.vector.scalar_tensor_tensor(
                        o[:], accs[ib][:, 0:OUT_DIM], recip[:], wb_bc[:],
                        op0=MULT, op1=ADD)
                    r0 = h * HALF + ib * 128
                    eng.dma_start(out_d[r0:r0 + 128, :], o[:])

                def group_mms(g, ib, xgrp, acc):
                    for ckl in range(GRP):
                        ck = g * GRP + ckl
                        nc.tensor.matmul(
                            acc[:],
                            xgrp[:, ckl, ib * 128:(ib + 1) * 128],
                            wh_aug[:, ck, 0:257],
                            start=(ck == 0), stop=(ck == NCK - 1),
                            skip_group_check=True)

                def accumulate(h, accs, prefetched, pre_masks,
                               drain_last=False):
                    for g in range(NG):
                        if h == 0 and g < NCK // 8 - 2:
                            build_wh_block(g + 2)
                        if h == 1 and g < len(prefetched):
                            xgrp = prefetched[g]
                        elif h == 0 and g < len(pre_masks):
                            xgrp = emit_x(h, g, "", 6, mgrp=pre_masks[g])
                        else:
                            xgrp = emit_x(h, g, "", 6)
                        if drain_last and g == NG - 1:
                            # ib-major for the final group: each acc bank
                            # finishes early and its drain overlaps the
                            # remaining matmuls
                            engs = [nc.sync, nc.scalar, nc.gpsimd, nc.sync]
                            for ib in range(4):
                                group_mms(g, ib, xgrp, accs[ib])
                                drain_one(h, ib, accs, engs[ib])
                        else:
                            for ckl in range(GRP):
                                ck = g * GRP + ckl
                                for ib in range(4):
                                    nc.tensor.matmul(
                                        accs[ib][:],
                                        xgrp[:, ckl, ib * 128:(ib + 1) * 128],
                                        wh_aug[:, ck, 0:257],
                                        start=(ck == 0), stop=(ck == NCK - 1),
                                        skip_group_check=True)

                def drain(h, accs):
                    engs = [nc.sync, nc.scalar, nc.sync, nc.scalar]
                    for ib in range(4):
                        drain_one(h, ib, accs, engs[ib])

                accs0 = [psB.tile([128, 257], F32, name=f"acc{ib}",
                                  tag=f"acc{ib}") for ib in range(4)]
                accumulate(0, accs0, [], pre_masks)
                # build h=1's first score groups while the DVE is idle at
                # the tail of h=0
                prefetched = [emit_x(1, 0, "", 6, xbufs=5),
                              emit_x(1, 1, "", 6, xbufs=5),
                              emit_x(1, 2, "", 6, xbufs=5),
                              emit_x(1, 3, "", 6, xbufs=5)]
                psA_stack.close()   # free pwh/pw1 PSUM banks for h1 accs
                with tc.tile_pool(name="psC", bufs=1, space="PSUM") as psC:
                    accs1 = [psC.tile([128, 257], F32, name=f"acc1{ib}",
                                      tag=f"acc1{ib}") for ib in range(4)]
                    drain(0, accs0)
                    accumulate(1, accs1, prefetched, pre_masks,
                               drain_last=True)
    nc.compile()
    return nc


def _get_nc():
    if "nc" not in _CACHE:
        _CACHE["nc"] = _build_nc()
    return _CACHE["nc"]


def _prep_in_maps(nodes, edge_mat, W_w, W_b, a1_w, a1_b, a2_w, a2_b):
    f16 = np.float16
    nodes = np.asarray(nodes, dtype=np.float32)
    edge_mat = np.asarray(edge_mat, dtype=bool)
    W_w = np.asarray(W_w, dtype=np.float32)
    W_b = np.asarray(W_b, dtype=np.float32)
    a1_w = np.asarray(a1_w, dtype=np.float32)
    a1_b = np.asarray(a1_b, dtype=np.float32)
    a2_w = np.asarray(a2_w, dtype=np.float32)
    a2_b = np.asarray(a2_b, dtype=np.float32)

    nodesT = np.ascontiguousarray(nodes.T).astype(f16)          # [512, 8192]
    v1 = (W_w.T @ a1_w[0]).astype(f16)[:, None]                 # [512, 1]
    v2 = (W_w.T @ a2_w[0]).astype(f16)[:, None]
    wt_aug = np.concatenate([W_w.T.astype(f16), v2, v1], axis=1)  # [512, 258]
    c1v = float(W_b @ a1_w[0]) + float(a1_b[0])
    c2v = float(W_b @ a2_w[0]) + float(a2_b[0])
    c2qr = np.broadcast_to(
        np.array([ALPHA * c2v, c2v, c1v], np.float32)[None, :],
        (128, 3)).copy()
    wb_bc = np.ascontiguousarray(
        np.broadcast_to(W_b[None, :], (128, OUT_DIM))).astype(np.float32)
    # multiplicative {0,1} mask, transposed, fp8 (cast to fp16 during DMA)
    import ml_dtypes
    maskT = np.where(edge_mat, 1, 0).astype(ml_dtypes.float8_e4m3fn).T

    in_maps = []
    for c in range(N_CORES):
        sl = slice(c * ROWS, (c + 1) * ROWS)
        # permute node columns so this core's own block comes first; the
        # j-order of Wh/X/mask rows follows the same permutation (softmax
        # sums over j are permutation-invariant, output rows stay i-ordered)
        nT = np.concatenate(
            [nodesT[:, sl], nodesT[:, :sl.start], nodesT[:, sl.stop:]],
            axis=1)
        mcol = maskT[:, sl]
        mperm = np.concatenate(
            [mcol[sl.start:sl.stop], mcol[:sl.start], mcol[sl.stop:]], axis=0)
        in_maps.append({
            "nodesT": np.ascontiguousarray(nT),
            "maskm": np.ascontiguousarray(mperm),
            "wt_aug": wt_aug,
            "wb_bc": wb_bc,
            "c2qr": c2qr,
        })
    return in_maps


def _run(inputs, trace=False, trace_cores=None):
    from concourse.bass_utils import run_bass_kernel_spmd
    if trace:
        _ensure_ntff_hook()
    nc = _get_nc()
    in_maps = _prep_in_maps(**inputs)
    res = run_bass_kernel_spmd(nc, in_maps, list(range(N_CORES)),
                               trace=trace, trace_cores=trace_cores)
    out = np.concatenate([res.results[c]["out"] for c in range(N_CORES)],
                         axis=0)
    return out, res


def kernel(**inputs) -> np.ndarray:
    out, _ = _run(inputs, trace=False)
    return out


# revision 31
# speedup vs baseline: 1.1865x; 1.0042x over previous
"""DenseGraphAttentionHead Trainium2 Bass kernel (8-core SPMD row-sharded).

reference math:
    Wh = nodes @ W_w.T + W_b                    [N, 256]
    Wh1 = Wh @ a1_w.T + a1_b                    [N, 1]
    Wh2 = Wh @ a2_w.T + a2_b                    [N, 1]
    scores = leaky_relu(Wh1 + Wh2.T, 0.2)       [N, N]
    attention = softmax(where(edge, scores, -inf), axis=1)
    out = attention @ Wh                        [N, 256]

Key identity: softmax over j is invariant to per-row(i) factors, so with
    p[i] = exp(0.8*Wh1[i]),  q[j] = exp(0.2*Wh2[j]),  r[j] = exp(Wh2[j])
we have  exp(lrelu(Wh1+Wh2) - 0.2*Wh1) = max(q[j], r[j]*p[i])
(branch r*p >= q  <=>  Wh1+Wh2 >= 0, exactly the lrelu branch), hence
    attention_ij ∝ edge_ij * max(q[j], r[j]*p[i]).
The dense exp/lrelu over the 8192x8192 score matrix collapses to one fused
DVE tensor_scalar (mult+max) per 128-chunk plus one tensor_tensor multiply
with the {0,1} edge mask (fp8 in HBM, upcast during the SWDGE DMA); exps
only run on vectors.

Per core c (rows i in [c*1024, (c+1)*1024), scores in [j(part), i(free)]):
  - Wh_aug[j, 0:256] = nodes @ W_w.T (fp16, no bias), col 256 = 1 (rowsum
    column), col 257 = nodes @ v2 = Wh2-c2 (a2 folded into params host-side).
  - X[j, i] = max(q[j], r[j]*p[i]) * mask01[j, i].
  - psum[i, 0:258] += X[:, i_blk].T @ Wh_aug over j chunks; col 256 = softmax
    denominator. out = psum[:, :256]/denom + W_b (softmax rows sum to 1, so
    the +W_b bias commutes with attention@).
"""
import sys
import types

import numpy as np

N_NODES = 8192
IN_DIM = 512
OUT_DIM = 256
ALPHA = 0.2
N_CORES = 8
ROWS = N_NODES // N_CORES          # 1024 rows per core
NCK = N_NODES // 128               # 64 j-chunks of 128
GRP = 8                            # j-chunks per mask-DMA batch

_CACHE = {}


def _ensure_ntff_hook():
    """antenv.axon_hooks is absent in this container; shim it so
    run_bass_kernel_spmd(trace=True) can reach the NTFF profiler."""
    if "antenv.axon_hooks" in sys.modules:
        return
    holder = [None]
    mod = types.ModuleType("antenv.axon_hooks")
    mod.set_axon_ntff_profile_hook = lambda h: holder.__setitem__(0, h)
    mod.get_axon_ntff_profile_hook = lambda: holder[0]
    sys.modules["antenv.axon_hooks"] = mod
    try:
        from trn_agent_boot.trn_boot import _ntff_profile_via_ctypes
        mod.set_axon_ntff_profile_hook(
            _ntff_profile_via_ctypes("/opt/axon/libaxon_pjrt.so"))
    except Exception:
        pass


def _register_gat_op():
    """Register a fused custom DVE op: out = max(in0*s0, s1) * in1.
    One 1x pass replaces the tensor_scalar + tensor_tensor pair and can
    emit fp8 directly. Uses the official per-NEFF custom-DVE table path."""
    from concourse import dve_ops
    from concourse.dve_spec import Spec, Src0, Src1, C0, C1, maxx, lower
    from concourse.dve_spec import _has_src1 as has_src1
    from concourse.dve_uop import DveOpSpec

    name = "GAT_SMAX_MASK"
    if name in dve_ops._SUB_OPCODE_FOR_NAME:
        return next(o for o in dve_ops.OPS if o.name == name)
    spec = Spec(
        body=maxx(Src0 * C0, C1) * Src1,
        reference=lambda in0, in1, s0, s1: np.maximum(in0 * s0, s1) * in1,
    )
    row = dve_ops._CUSTOM_DVE_ROW_BASE + len(dve_ops.OPS)
    shas = {}
    for ver in ("v3", "v4"):
        tmp = DveOpSpec(name=name, opcode=row, uops=lower(spec, ver=ver),
                        rd1_en=has_src1(spec))
        shas[ver] = tmp.sha(ver)
    op = dve_ops.DveOp(name, spec, subdim=False, uops_sha=shas)
    dve_ops.OPS.append(op)
    dve_ops._SUB_OPCODE_FOR_NAME[name] = row
    return op


# X-production strategy per (half, group); default "dve2" (DVE ts+tt).
# "act" offloads the score op of a group to the Act engine (relu identity:
# max(q, r*p) = relu(r*p - q) + q) so the DVE only runs one combine pass.
# Measured: Pool-engine offload and fp8 DoubleRow are both net losses
# (SBUF/queue contention; DR spacing 214ns vs 2x112ns bf16).
X_VARIANTS = {}


def _build_nc():
    import concourse.bacc as bacc
    import concourse.tile as tile
    from concourse import mybir

    gat_op = _register_gat_op()

    F16 = mybir.dt.float16
    BF16 = mybir.dt.bfloat16
    F32 = mybir.dt.float32
    FP8 = mybir.dt.float8e4
    ADD = mybir.AluOpType.add
    MULT = mybir.AluOpType.mult
    MAX = mybir.AluOpType.max
    EXP = mybir.ActivationFunctionType.Exp
    RELU = mybir.ActivationFunctionType.Relu
    DR = mybir.MatmulPerfMode.DoubleRow

    nc = bacc.Bacc("TRN2", target_bir_lowering=False, debug=False,
                   num_devices=N_CORES)

    nodesT_d = nc.dram_tensor("nodesT", [IN_DIM, N_NODES], F16,
                              kind="ExternalInput")
    maskm_d = nc.dram_tensor("maskm", [N_NODES, ROWS], mybir.dt.float8e4,
                             kind="ExternalInput")
    # cols 0:256 = W_w.T, col 256 = v2 (a2 folded), col 257 = v1 (a1 folded)
    wtaug_d = nc.dram_tensor("wt_aug", [IN_DIM, 258], F16,
                             kind="ExternalInput")
    wb_d = nc.dram_tensor("wb_bc", [128, OUT_DIM], F32, kind="ExternalInput")
    # col 0 = 0.2*c2 (q bias), col 1 = 0.8*c2 (rt bias), col 2 = c1
    c2_d = nc.dram_tensor("c2qr", [128, 3], F32, kind="ExternalInput")
    out_d = nc.dram_tensor("out", [ROWS, OUT_DIM], F32, kind="ExternalOutput")

    with tile.TileContext(nc) as tc:
        with (
            tc.tile_pool(name="consts", bufs=1) as consts,
            tc.tile_pool(name="ndpool", bufs=5) as ndpool,
            tc.tile_pool(name="grpp", bufs=4) as grpp,
            tc.tile_pool(name="outp", bufs=2) as outp,
        ):
            # ---- constants. Critical path: ndT0 + wt gate pw1 -> p_row
            # and the Wh build. One batched DMA per tensor, spread over
            # queues so nothing serializes behind bulk traffic.
            # block 0 as two half tiles: pw1/h0 and build chunks 0-3 start
            # as soon as the first 512 columns land
            ndT0h = []
            for h2 in range(2):
                t = ndpool.tile([128, 4, 512], F16, name="ndT0", tag="ndT0",
                                bufs=2)
                nd_src = nodesT_d[:, h2 * 512:(h2 + 1) * 512]
                nd_src = nd_src.rearrange("(d p) i -> p d i", p=128)
                nc.sync.dma_start(t[:], nd_src)
                ndT0h.append(t)

            wt = consts.tile([128, 4, 258], F16, name="wt", tag="wt")
            nc.scalar.dma_start(
                wt[:], wtaug_d.rearrange("(d p) f -> p d f", p=128))
            wt_t = [wt[:, d4, 0:257] for d4 in range(4)]
            v1_t = [wt[:, d4, 257:258] for d4 in range(4)]

            c2qr = consts.tile([128, 3], F32)
            nc.gpsimd.dma_start(c2qr[:], c2_d[:])
            c1 = c2qr[0:1, 2:3]
            wb_bc = consts.tile([128, OUT_DIM], F32)
            nc.gpsimd.dma_start(wb_bc[:], wb_d[:])

            # wh_aug chunk layout: cols 0:256 = Wh (bf16), col 256 = 1.0
            # (softmax-denominator column, memset once below; the per-chunk
            # copies never touch it). Wh2 never lands in SBUF wh_aug; it is
            # extracted straight from PSUM col 256 of each chunk matmul.
            wh_aug = consts.tile([128, NCK, 257], BF16)
            nc.gpsimd.memset(wh_aug[:, :, 256:257], 1.0)
            wh2f32 = consts.tile([128, NCK], F32)
            q128 = consts.tile([128, NCK], F32)
            nq128 = consts.tile([128, NCK], F32)
            r128 = consts.tile([128, NCK], F32)

            HALF = 512
            NG = NCK // GRP            # 16 groups of GRP chunks per half
            import contextlib
            psA_stack = contextlib.ExitStack()
            with tc.tile_pool(name="psB", bufs=1, space="PSUM") as psB:
                psA = psA_stack.enter_context(
                    tc.tile_pool(name="psA", bufs=2, space="PSUM"))
                # ---- Wh1 row for own block + p = exp(0.8*Wh1) broadcast ----
                # pw1 borrows a pwh2-pool buffer (PSUM budget: 2x2 pwh2
                # banks + 4 acc banks = 8)
                wh1row = consts.tile([1, ROWS], F16)
                for h2 in range(2):
                    pw1t = psA.tile([128, 2, 512], F32, name="pwh2",
                                    tag="pwh2", bufs=2)
                    pw1 = pw1t[0:1, 0, :]
                    for d4 in range(4):
                        nc.tensor.matmul(
                            pw1, v1_t[d4], ndT0h[h2][:, d4, :],
                            start=(d4 == 0), stop=(d4 == 3),
                            skip_group_check=True)
                    nc.vector.tensor_scalar(
                        wh1row[:, h2 * 512:(h2 + 1) * 512], pw1, c1[:],
                        None, op0=ADD)
                p_row = consts.tile([1, ROWS], BF16)
                nc.scalar.activation(p_row[:], wh1row[:], EXP, scale=ALPHA * 4)

                p_b = consts.tile([128, ROWS], BF16)

                def build_wh_block(b, ndT=None):
                    if b != 0 and ndT is None:
                        ndT = ndpool.tile([128, 4, 1024], F16, name="ndT",
                                          tag="ndT")
                        # alternate queues so ndT transfers pipeline two
                        # at a time
                        dma_eng = nc.scalar if b % 2 == 1 else nc.sync
                        nd_src = nodesT_d[:, b * 1024:(b + 1) * 1024]
                        nd_src = nd_src.rearrange("(d p) i -> p d i", p=128)
                        dma_eng.dma_start(ndT[:], nd_src)

                    def nd_ap(ckl, d4):
                        if b == 0:
                            h2, c4 = ckl // 4, ckl % 4
                            return ndT0h[h2][:, d4, c4 * 128:(c4 + 1) * 128]
                        return ndT[:, d4, ckl * 128:(ckl + 1) * 128]

                    for cp in range(4):
                        ck0 = b * 8 + cp * 2
                        # chunk pair in one 2-bank PSUM tile -> batched
                        # (2-chunk) extracts and copies halve ACT overhead
                        pwh2 = psA.tile([128, 2, 512], F32, name="pwh2",
                                        tag="pwh2", bufs=2)
                        for k2 in range(2):
                            ckl = cp * 2 + k2
                            for d4 in range(4):
                                nc.tensor.matmul(
                                    pwh2[:, k2, 0:257], nd_ap(ckl, d4),
                                    wt_t[d4],
                                    start=(d4 == 0), stop=(d4 == 3),
                                    skip_group_check=True)
                        # wh2 extract first (feeds q/r -> X pipeline); DVE
                        # for the first blocks where it is idle, ACT after
                        if b < 2:
                            nc.vector.tensor_copy(wh2f32[:, ck0:ck0 + 2],
                                                  pwh2[:, :, 256:257])
                        else:
                            nc.scalar.copy(wh2f32[:, ck0:ck0 + 2],
                                           pwh2[:, :, 256:257])
                        if cp == 3:
                            sl = slice(b * 8, (b + 1) * 8)
                            nc.scalar.activation(q128[:, sl], wh2f32[:, sl],
                                                 EXP, scale=ALPHA,
                                                 bias=c2qr[:, 0:1])
                            nc.scalar.activation(r128[:, sl], wh2f32[:, sl],
                                                 EXP, scale=1.0,
                                                 bias=c2qr[:, 1:2])
                            nc.scalar.mul(nq128[:, sl], q128[:, sl], -1.0)

                        nc.scalar.copy(wh_aug[:, ck0:ck0 + 2, 0:256],
                                       pwh2[:, :, 0:256])

                build_wh_block(0)
                build_wh_block(1)

                # ---- main sweep over i-halves, Wh blocks interleaved in
                # PE program order during the first half ----
                def emit_mask(h, g, suffix, bufs, dt=BF16):
                    mgrp = grpp.tile([128, GRP, HALF], dt,
                                     name=f"mgrp{suffix}",
                                     tag=f"mgrp{suffix}", bufs=bufs)
                    msrc = maskm_d[g * GRP * 128:(g + 1) * GRP * 128,
                                   h * HALF:(h + 1) * HALF]
                    msrc = msrc.rearrange("(c p) i -> p c i", p=128)
                    nc.gpsimd.dma_start(mgrp[:], msrc)  # fp8->bf16 cast
                    return mgrp

                def emit_x(h, g, suffix, bufs, xbufs=None, mgrp=None):
                    variant = X_VARIANTS.get((h, g), "dve2")
                    is8 = variant.endswith("8")
                    base = variant[:-1] if is8 else variant
                    if mgrp is None:
                        mgrp = emit_mask(h, g, suffix, bufs)
                    xtag = f"xgrp{suffix}" + ("8" if is8 else "")
                    xgrp = grpp.tile([128, GRP, HALF], FP8 if is8 else BF16,
                                     name=xtag, tag=xtag,
                                     bufs=xbufs or 3)
                    pb_h = p_b[:, h * HALF:(h + 1) * HALF]
                    if base == "fused":
                        for ckl in range(GRP):
                            ck = g * GRP + ckl
                            nc.vector._custom_dve(
                                gat_op, out=xgrp[:, ckl, :], in0=pb_h,
                                in1=mgrp[:, ckl, :],
                                s0=r128[:, ck:ck + 1], s1=q128[:, ck:ck + 1])
                        return xgrp
                    if base == "act":
                        agrp = grpp.tile([128, GRP, HALF], BF16,
                                         name=f"agrp{suffix}",
                                         tag=f"agrp{suffix}", bufs=3)
                        for ckl in range(GRP):
                            ck = g * GRP + ckl
                            nc.scalar.activation(
                                agrp[:, ckl, :], pb_h, RELU,
                                scale=r128[:, ck:ck + 1],
                                bias=nq128[:, ck:ck + 1])
                            nc.vector.scalar_tensor_tensor(
                                xgrp[:, ckl, :], agrp[:, ckl, :],
                                q128[:, ck:ck + 1], mgrp[:, ckl, :],
                                op0=ADD, op1=MULT)
                        return xgrp
                    sgrp = grpp.tile([128, GRP, HALF], BF16,
                                     name=f"sgrp{suffix}",
                                     tag=f"sgrp{suffix}", bufs=3)
                    for ckl in range(GRP):
                        ck = g * GRP + ckl
                        nc.vector.tensor_scalar(
                            sgrp[:, ckl, :], pb_h,
                            r128[:, ck:ck + 1], q128[:, ck:ck + 1],
                            op0=MULT, op1=MAX)
                    if base == "pool":
                        nc.gpsimd.tensor_tensor(xgrp[:], sgrp[:], mgrp[:],
                                                op=MULT)
                    else:
                        nc.vector.tensor_tensor(xgrp[:], sgrp[:], mgrp[:],
                                                op=MULT)
                    return xgrp

                pre_masks = [emit_mask(0, 0, "", 4),
                             emit_mask(0, 1, "", 4)]
                nc.gpsimd.partition_broadcast(p_b[:], p_row[:])

                def drain_one(h, ib, accs, eng):
                    recip = outp.tile([128, 1], F32, name="recip",
                                      tag="recip", bufs=4)
                    nc.vector.reciprocal(recip[:], accs[ib][:, 256:257])
                    o = outp.tile([128, OUT_DIM], F32, name="o", tag="o",
                                  bufs=4)
                    nc.vector.scalar_tensor_tensor(
                        o[:], accs[ib][:, 0:OUT_DIM], recip[:], wb_bc[:],
                        op0=MULT, op1=ADD)
                    r0 = h * HALF + ib * 128
                    eng.dma_start(out_d[r0:r0 + 128, :], o[:])

                def group_mms(g, ib, xgrp, acc):
                    for ckl in range(GRP):
                        ck = g * GRP + ckl
                        nc.tensor.matmul(
                            acc[:],
                            xgrp[:, ckl, ib * 128:(ib + 1) * 128],
                            wh_aug[:, ck, 0:257],
                            start=(ck == 0), stop=(ck == NCK - 1),
                            skip_group_check=True)

                def accumulate(h, accs, prefetched, pre_masks,
                               drain_last=False):
                    for g in range(NG):
                        if h == 0 and g < NCK // 8 - 2:
                            build_wh_block(g + 2)
                        if h == 0 and g < NCK // 8 - 2 - NG:
                            build_wh_block(g + 2 + NG)
                        if h == 1 and g < len(prefetched):
                            xgrp = prefetched[g]
                        elif h == 0 and g < len(pre_masks):
                            xgrp = emit_x(h, g, "", 4, mgrp=pre_masks[g])
                        else:
                            xgrp = emit_x(h, g, "", 4)
                        if drain_last and g == NG - 1:
                            # ib-major for the final group: each acc bank
                            # finishes early and its drain overlaps the
                            # remaining matmuls
                            engs = [nc.sync, nc.scalar, nc.gpsimd, nc.sync]
                            for ib in range(4):
                                group_mms(g, ib, xgrp, accs[ib])
                                drain_one(h, ib, accs, engs[ib])
                        else:
                            for ckl in range(GRP):
                                ck = g * GRP + ckl
                                for ib in range(4):
                                    nc.tensor.matmul(
                                        accs[ib][:],
                                        xgrp[:, ckl, ib * 128:(ib + 1) * 128],
                                        wh_aug[:, ck, 0:257],
                                        start=(ck == 0), stop=(ck == NCK - 1),
                                        skip_group_check=True)

                def drain(h, accs):
                    engs = [nc.sync, nc.scalar, nc.sync, nc.scalar]
                    for ib in range(4):
                        drain_one(h, ib, accs, engs[ib])

                accs0 = [psB.tile([128, 257], F32, name=f"acc{ib}",
                                  tag=f"acc{ib}") for ib in range(4)]
                accumulate(0, accs0, [], pre_masks)
                # build h=1's first score groups while the DVE is idle at
                # the tail of h=0
                prefetched = [emit_x(1, 0, "", 4, xbufs=3),
                              emit_x(1, 1, "", 4, xbufs=3)]
                psA_stack.close()   # free pwh/pw1 PSUM banks for h1 accs
                with tc.tile_pool(name="psC", bufs=1, space="PSUM") as psC:
                    accs1 = [psC.tile([128, 257], F32, name=f"acc1{ib}",
                                      tag=f"acc1{ib}") for ib in range(4)]
                    drain(0, accs0)
                    accumulate(1, accs1, prefetched, pre_masks,
                               drain_last=True)
    nc.compile()
    return nc


def _get_nc():
    if "nc" not in _CACHE:
        _CACHE["nc"] = _build_nc()
    return _CACHE["nc"]


def _prep_in_maps(nodes, edge_mat, W_w, W_b, a1_w, a1_b, a2_w, a2_b):
    f16 = np.float16
    nodes = np.asarray(nodes, dtype=np.float32)
    edge_mat = np.asarray(edge_mat, dtype=bool)
    W_w = np.asarray(W_w, dtype=np.float32)
    W_b = np.asarray(W_b, dtype=np.float32)
    a1_w = np.asarray(a1_w, dtype=np.float32)
    a1_b = np.asarray(a1_b, dtype=np.float32)
    a2_w = np.asarray(a2_w, dtype=np.float32)
    a2_b = np.asarray(a2_b, dtype=np.float32)

    nodesT = np.ascontiguousarray(nodes.T).astype(f16)          # [512, 8192]
    v1 = (W_w.T @ a1_w[0]).astype(f16)[:, None]                 # [512, 1]
    v2 = (W_w.T @ a2_w[0]).astype(f16)[:, None]
    wt_aug = np.concatenate([W_w.T.astype(f16), v2, v1], axis=1)  # [512, 258]
    c1v = float(W_b @ a1_w[0]) + float(a1_b[0])
    c2v = float(W_b @ a2_w[0]) + float(a2_b[0])
    c2qr = np.broadcast_to(
        np.array([ALPHA * c2v, c2v, c1v], np.float32)[None, :],
        (128, 3)).copy()
    wb_bc = np.ascontiguousarray(
        np.broadcast_to(W_b[None, :], (128, OUT_DIM))).astype(np.float32)
    # multiplicative {0,1} mask, transposed, fp8 (cast to fp16 during DMA)
    import ml_dtypes
    maskT = np.where(edge_mat, 1, 0).astype(ml_dtypes.float8_e4m3fn).T

    in_maps = []
    for c in range(N_CORES):
        sl = slice(c * ROWS, (c + 1) * ROWS)
        # permute node columns so this core's own block comes first; the
        # j-order of Wh/X/mask rows follows the same permutation (softmax
        # sums over j are permutation-invariant, output rows stay i-ordered)
        nT = np.concatenate(
            [nodesT[:, sl], nodesT[:, :sl.start], nodesT[:, sl.stop:]],
            axis=1)
        mcol = maskT[:, sl]
        mperm = np.concatenate(
            [mcol[sl.start:sl.stop], mcol[:sl.start], mcol[sl.stop:]], axis=0)
        in_maps.append({
            "nodesT": np.ascontiguousarray(nT),
            "maskm": np.ascontiguousarray(mperm),
            "wt_aug": wt_aug,
            "wb_bc": wb_bc,
            "c2qr": c2qr,
        })
    return in_maps


def _run(inputs, trace=False, trace_cores=None):
    from concourse.bass_utils import run_bass_kernel_spmd
    if trace:
        _ensure_ntff_hook()
    nc = _get_nc()
    in_maps = _prep_in_maps(**inputs)
    res = run_bass_kernel_spmd(nc, in_maps, list(range(N_CORES)),
                               trace=trace, trace_cores=trace_cores)
    out = np.concatenate([res.results[c]["out"] for c in range(N_CORES)],
                         axis=0)
    return out, res


def kernel(**inputs) -> np.ndarray:
    out, _ = _run(inputs, trace=False)
    return out


# revision 32
# speedup vs baseline: 1.2671x; 1.0679x over previous
"""DenseGraphAttentionHead Trainium2 Bass kernel (8-core SPMD row-sharded).

reference math:
    Wh = nodes @ W_w.T + W_b                    [N, 256]
    Wh1 = Wh @ a1_w.T + a1_b                    [N, 1]
    Wh2 = Wh @ a2_w.T + a2_b                    [N, 1]
    scores = leaky_relu(Wh1 + Wh2.T, 0.2)       [N, N]
    attention = softmax(where(edge, scores, -inf), axis=1)
    out = attention @ Wh                        [N, 256]

Key identity: softmax over j is invariant to per-row(i) factors, so with
    p[i] = exp(0.8*Wh1[i]),  q[j] = exp(0.2*Wh2[j]),  r[j] = exp(Wh2[j])
we have  exp(lrelu(Wh1+Wh2) - 0.2*Wh1) = max(q[j], r[j]*p[i])
(branch r*p >= q  <=>  Wh1+Wh2 >= 0, exactly the lrelu branch), hence
    attention_ij ∝ edge_ij * max(q[j], r[j]*p[i]).
The dense exp/lrelu over the 8192x8192 score matrix collapses to one fused
DVE tensor_scalar (mult+max) per 128-chunk plus one tensor_tensor multiply
with the {0,1} edge mask (fp8 in HBM, upcast during the SWDGE DMA); exps
only run on vectors.

Per core c (rows i in [c*1024, (c+1)*1024), scores in [j(part), i(free)]):
  - Wh_aug[j, 0:256] = nodes @ W_w.T (fp16, no bias), col 256 = 1 (rowsum
    column), col 257 = nodes @ v2 = Wh2-c2 (a2 folded into params host-side).
  - X[j, i] = max(q[j], r[j]*p[i]) * mask01[j, i].
  - psum[i, 0:258] += X[:, i_blk].T @ Wh_aug over j chunks; col 256 = softmax
    denominator. out = psum[:, :256]/denom + W_b (softmax rows sum to 1, so
    the +W_b bias commutes with attention@).
"""
import sys
import types

import numpy as np

N_NODES = 8192
IN_DIM = 512
OUT_DIM = 256
ALPHA = 0.2
N_CORES = 8
ROWS = N_NODES // N_CORES          # 1024 rows per core
NCK = N_NODES // 128               # 64 j-chunks of 128
GRP = 4                            # j-chunks per mask-DMA batch

_CACHE = {}


def _ensure_ntff_hook():
    """antenv.axon_hooks is absent in this container; shim it so
    run_bass_kernel_spmd(trace=True) can reach the NTFF profiler."""
    if "antenv.axon_hooks" in sys.modules:
        return
    holder = [None]
    mod = types.ModuleType("antenv.axon_hooks")
    mod.set_axon_ntff_profile_hook = lambda h: holder.__setitem__(0, h)
    mod.get_axon_ntff_profile_hook = lambda: holder[0]
    sys.modules["antenv.axon_hooks"] = mod
    try:
        from trn_agent_boot.trn_boot import _ntff_profile_via_ctypes
        mod.set_axon_ntff_profile_hook(
            _ntff_profile_via_ctypes("/opt/axon/libaxon_pjrt.so"))
    except Exception:
        pass


def _register_gat_op():
    """Register a fused custom DVE op: out = max(in0*s0, s1) * in1.
    One 1x pass replaces the tensor_scalar + tensor_tensor pair and can
    emit fp8 directly. Uses the official per-NEFF custom-DVE table path."""
    from concourse import dve_ops
    from concourse.dve_spec import Spec, Src0, Src1, C0, C1, maxx, lower
    from concourse.dve_spec import _has_src1 as has_src1
    from concourse.dve_uop import DveOpSpec

    name = "GAT_SMAX_MASK"
    if name in dve_ops._SUB_OPCODE_FOR_NAME:
        return next(o for o in dve_ops.OPS if o.name == name)
    spec = Spec(
        body=maxx(Src0 * C0, C1) * Src1,
        reference=lambda in0, in1, s0, s1: np.maximum(in0 * s0, s1) * in1,
    )
    row = dve_ops._CUSTOM_DVE_ROW_BASE + len(dve_ops.OPS)
    shas = {}
    for ver in ("v3", "v4"):
        tmp = DveOpSpec(name=name, opcode=row, uops=lower(spec, ver=ver),
                        rd1_en=has_src1(spec))
        shas[ver] = tmp.sha(ver)
    op = dve_ops.DveOp(name, spec, subdim=False, uops_sha=shas)
    dve_ops.OPS.append(op)
    dve_ops._SUB_OPCODE_FOR_NAME[name] = row
    return op


# X-production strategy per (half, group); default "dve2" (DVE ts+tt).
# "act" offloads the score op of a group to the Act engine (relu identity:
# max(q, r*p) = relu(r*p - q) + q) so the DVE only runs one combine pass.
# Measured: Pool-engine offload and fp8 DoubleRow are both net losses
# (SBUF/queue contention; DR spacing 214ns vs 2x112ns bf16).
X_VARIANTS = {}


def _build_nc():
    import concourse.bacc as bacc
    import concourse.tile as tile
    from concourse import mybir

    gat_op = _register_gat_op()

    F16 = mybir.dt.float16
    BF16 = mybir.dt.bfloat16
    F32 = mybir.dt.float32
    FP8 = mybir.dt.float8e4
    ADD = mybir.AluOpType.add
    MULT = mybir.AluOpType.mult
    MAX = mybir.AluOpType.max
    EXP = mybir.ActivationFunctionType.Exp
    RELU = mybir.ActivationFunctionType.Relu
    DR = mybir.MatmulPerfMode.DoubleRow

    nc = bacc.Bacc("TRN2", target_bir_lowering=False, debug=False,
                   num_devices=N_CORES)

    nodesT_d = nc.dram_tensor("nodesT", [IN_DIM, N_NODES], F16,
                              kind="ExternalInput")
    maskm_d = nc.dram_tensor("maskm", [N_NODES, ROWS], mybir.dt.float8e4,
                             kind="ExternalInput")
    # cols 0:256 = W_w.T, col 256 = v2 (a2 folded), col 257 = v1 (a1 folded)
    wtaug_d = nc.dram_tensor("wt_aug", [IN_DIM, 258], F16,
                             kind="ExternalInput")
    wb_d = nc.dram_tensor("wb_bc", [128, OUT_DIM], F32, kind="ExternalInput")
    # col 0 = 0.2*c2 (q bias), col 1 = 0.8*c2 (rt bias), col 2 = c1
    c2_d = nc.dram_tensor("c2qr", [128, 3], F32, kind="ExternalInput")
    out_d = nc.dram_tensor("out", [ROWS, OUT_DIM], F32, kind="ExternalOutput")

    with tile.TileContext(nc) as tc:
        with (
            tc.tile_pool(name="consts", bufs=1) as consts,
            tc.tile_pool(name="ndpool", bufs=5) as ndpool,
            tc.tile_pool(name="grpp", bufs=4) as grpp,
            tc.tile_pool(name="outp", bufs=2) as outp,
        ):
            # ---- constants. Critical path: ndT0 + wt gate pw1 -> p_row
            # and the Wh build. One batched DMA per tensor, spread over
            # queues so nothing serializes behind bulk traffic.
            # block 0 as two half tiles: pw1/h0 and build chunks 0-3 start
            # as soon as the first 512 columns land
            ndT0h = []
            for h2 in range(2):
                t = ndpool.tile([128, 4, 512], F16, name="ndT0", tag="ndT0",
                                bufs=2)
                nd_src = nodesT_d[:, h2 * 512:(h2 + 1) * 512]
                nd_src = nd_src.rearrange("(d p) i -> p d i", p=128)
                nc.sync.dma_start(t[:], nd_src)
                ndT0h.append(t)

            wt = consts.tile([128, 4, 258], F16, name="wt", tag="wt")
            nc.scalar.dma_start(
                wt[:], wtaug_d.rearrange("(d p) f -> p d f", p=128))
            wt_t = [wt[:, d4, 0:257] for d4 in range(4)]
            v1_t = [wt[:, d4, 257:258] for d4 in range(4)]

            c2qr = consts.tile([128, 3], F32)
            nc.gpsimd.dma_start(c2qr[:], c2_d[:])
            c1 = c2qr[0:1, 2:3]
            wb_bc = consts.tile([128, OUT_DIM], F32)
            nc.gpsimd.dma_start(wb_bc[:], wb_d[:])

            # wh_aug chunk layout: cols 0:256 = Wh (bf16), col 256 = 1.0
            # (softmax-denominator column, memset once below; the per-chunk
            # copies never touch it). Wh2 never lands in SBUF wh_aug; it is
            # extracted straight from PSUM col 256 of each chunk matmul.
            wh_aug = consts.tile([128, NCK, 257], BF16)
            nc.gpsimd.memset(wh_aug[:, :, 256:257], 1.0)
            wh2f32 = consts.tile([128, NCK], F32)
            q128 = consts.tile([128, NCK], F32)
            nq128 = consts.tile([128, NCK], F32)
            r128 = consts.tile([128, NCK], F32)

            HALF = 512
            NG = NCK // GRP            # 16 groups of GRP chunks per half
            import contextlib
            psA_stack = contextlib.ExitStack()
            with tc.tile_pool(name="psB", bufs=1, space="PSUM") as psB:
                psA = psA_stack.enter_context(
                    tc.tile_pool(name="psA", bufs=2, space="PSUM"))
                # ---- Wh1 row for own block + p = exp(0.8*Wh1) broadcast ----
                # pw1 borrows a pwh2-pool buffer (PSUM budget: 2x2 pwh2
                # banks + 4 acc banks = 8)
                wh1row = consts.tile([1, ROWS], F16)
                for h2 in range(2):
                    pw1t = psA.tile([128, 2, 512], F32, name="pwh2",
                                    tag="pwh2", bufs=2)
                    pw1 = pw1t[0:1, 0, :]
                    for d4 in range(4):
                        nc.tensor.matmul(
                            pw1, v1_t[d4], ndT0h[h2][:, d4, :],
                            start=(d4 == 0), stop=(d4 == 3),
                            skip_group_check=True)
                    nc.vector.tensor_scalar(
                        wh1row[:, h2 * 512:(h2 + 1) * 512], pw1, c1[:],
                        None, op0=ADD)
                p_row = consts.tile([1, ROWS], BF16)
                nc.scalar.activation(p_row[:], wh1row[:], EXP, scale=ALPHA * 4)

                p_b = consts.tile([128, ROWS], BF16)

                def build_wh_block(b, ndT=None):
                    if b != 0 and ndT is None:
                        ndT = ndpool.tile([128, 4, 1024], F16, name="ndT",
                                          tag="ndT")
                        # alternate queues so ndT transfers pipeline two
                        # at a time
                        dma_eng = nc.scalar if b % 2 == 1 else nc.sync
                        nd_src = nodesT_d[:, b * 1024:(b + 1) * 1024]
                        nd_src = nd_src.rearrange("(d p) i -> p d i", p=128)
                        dma_eng.dma_start(ndT[:], nd_src)

                    def nd_ap(ckl, d4):
                        if b == 0:
                            h2, c4 = ckl // 4, ckl % 4
                            return ndT0h[h2][:, d4, c4 * 128:(c4 + 1) * 128]
                        return ndT[:, d4, ckl * 128:(ckl + 1) * 128]

                    for cp in range(4):
                        ck0 = b * 8 + cp * 2
                        # chunk pair in one 2-bank PSUM tile -> batched
                        # (2-chunk) extracts and copies halve ACT overhead
                        pwh2 = psA.tile([128, 2, 512], F32, name="pwh2",
                                        tag="pwh2", bufs=2)
                        for k2 in range(2):
                            ckl = cp * 2 + k2
                            for d4 in range(4):
                                nc.tensor.matmul(
                                    pwh2[:, k2, 0:257], nd_ap(ckl, d4),
                                    wt_t[d4],
                                    start=(d4 == 0), stop=(d4 == 3),
                                    skip_group_check=True)
                        # wh2 extract first (feeds q/r -> X pipeline); DVE
                        # for the first blocks where it is idle, ACT after
                        if b < 2:
                            nc.vector.tensor_copy(wh2f32[:, ck0:ck0 + 2],
                                                  pwh2[:, :, 256:257])
                        else:
                            nc.scalar.copy(wh2f32[:, ck0:ck0 + 2],
                                           pwh2[:, :, 256:257])
                        if cp == 3:
                            sl = slice(b * 8, (b + 1) * 8)
                            nc.scalar.activation(q128[:, sl], wh2f32[:, sl],
                                                 EXP, scale=ALPHA,
                                                 bias=c2qr[:, 0:1])
                            nc.scalar.activation(r128[:, sl], wh2f32[:, sl],
                                                 EXP, scale=1.0,
                                                 bias=c2qr[:, 1:2])
                            nc.scalar.mul(nq128[:, sl], q128[:, sl], -1.0)

                        nc.scalar.copy(wh_aug[:, ck0:ck0 + 2, 0:256],
                                       pwh2[:, :, 0:256])

                build_wh_block(0)
                build_wh_block(1)

                # ---- main sweep over i-halves, Wh blocks interleaved in
                # PE program order during the first half ----
                def emit_mask(h, g, suffix, bufs, dt=BF16):
                    mgrp = grpp.tile([128, GRP, HALF], dt,
                                     name=f"mgrp{suffix}",
                                     tag=f"mgrp{suffix}", bufs=bufs)
                    msrc = maskm_d[g * GRP * 128:(g + 1) * GRP * 128,
                                   h * HALF:(h + 1) * HALF]
                    msrc = msrc.rearrange("(c p) i -> p c i", p=128)
                    nc.gpsimd.dma_start(mgrp[:], msrc)  # fp8->bf16 cast
                    return mgrp

                def emit_x(h, g, suffix, bufs, xbufs=None, mgrp=None):
                    variant = X_VARIANTS.get((h, g), "dve2")
                    is8 = variant.endswith("8")
                    base = variant[:-1] if is8 else variant
                    if mgrp is None:
                        mgrp = emit_mask(h, g, suffix, bufs)
                    xtag = f"xgrp{suffix}" + ("8" if is8 else "")
                    xgrp = grpp.tile([128, GRP, HALF], FP8 if is8 else BF16,
                                     name=xtag, tag=xtag,
                                     bufs=xbufs or 5)
                    pb_h = p_b[:, h * HALF:(h + 1) * HALF]
                    if base == "fused":
                        for ckl in range(GRP):
                            ck = g * GRP + ckl
                            nc.vector._custom_dve(
                                gat_op, out=xgrp[:, ckl, :], in0=pb_h,
                                in1=mgrp[:, ckl, :],
                                s0=r128[:, ck:ck + 1], s1=q128[:, ck:ck + 1])
                        return xgrp
                    if base == "act":
                        agrp = grpp.tile([128, GRP, HALF], BF16,
                                         name=f"agrp{suffix}",
                                         tag=f"agrp{suffix}", bufs=3)
                        for ckl in range(GRP):
                            ck = g * GRP + ckl
                            nc.scalar.activation(
                                agrp[:, ckl, :], pb_h, RELU,
                                scale=r128[:, ck:ck + 1],
                                bias=nq128[:, ck:ck + 1])
                            nc.vector.scalar_tensor_tensor(
                                xgrp[:, ckl, :], agrp[:, ckl, :],
                                q128[:, ck:ck + 1], mgrp[:, ckl, :],
                                op0=ADD, op1=MULT)
                        return xgrp
                    sgrp = grpp.tile([128, GRP, HALF], BF16,
                                     name=f"sgrp{suffix}",
                                     tag=f"sgrp{suffix}", bufs=4)
                    for ckl in range(GRP):
                        ck = g * GRP + ckl
                        nc.vector.tensor_scalar(
                            sgrp[:, ckl, :], pb_h,
                            r128[:, ck:ck + 1], q128[:, ck:ck + 1],
                            op0=MULT, op1=MAX)
                    if base == "pool":
                        nc.gpsimd.tensor_tensor(xgrp[:], sgrp[:], mgrp[:],
                                                op=MULT)
                    else:
                        nc.vector.tensor_tensor(xgrp[:], sgrp[:], mgrp[:],
                                                op=MULT)
                    return xgrp

                pre_masks = [emit_mask(0, 0, "", 6),
                             emit_mask(0, 1, "", 6),
                             emit_mask(0, 2, "", 6)]
                nc.gpsimd.partition_broadcast(p_b[:], p_row[:])

                def drain_one(h, ib, accs, eng):
                    recip = outp.tile([128, 1], F32, name="recip",
                                      tag="recip", bufs=4)
                    nc.vector.reciprocal(recip[:], accs[ib][:, 256:257])
                    o = outp.tile([128, OUT_DIM], F32, name="o", tag="o",
                                  bufs=4)
                    nc.vector.scalar_tensor_tensor(
                        o[:], accs[ib][:, 0:OUT_DIM], recip[:], wb_bc[:],
                        op0=MULT, op1=ADD)
                    r0 = h * HALF + ib * 128
                    eng.dma_start(out_d[r0:r0 + 128, :], o[:])

                def group_mms(g, ib, xgrp, acc):
                    for ckl in range(GRP):
                        ck = g * GRP + ckl
                        nc.tensor.matmul(
                            acc[:],
                            xgrp[:, ckl, ib * 128:(ib + 1) * 128],
                            wh_aug[:, ck, 0:257],
                            start=(ck == 0), stop=(ck == NCK - 1),
                            skip_group_check=True)

                def accumulate(h, accs, prefetched, pre_masks,
                               drain_last=False):
                    for g in range(NG):
                        if h == 0 and g < NCK // 8 - 2:
                            build_wh_block(g + 2)
                        if h == 0 and g < NCK // 8 - 2 - NG:
                            build_wh_block(g + 2 + NG)
                        if h == 1 and g < len(prefetched):
                            xgrp = prefetched[g]
                        elif h == 0 and g < len(pre_masks):
                            xgrp = emit_x(h, g, "", 6, mgrp=pre_masks[g])
                        else:
                            xgrp = emit_x(h, g, "", 6)
                        if drain_last and g == NG - 1:
                            # ib-major for the final group: each acc bank
                            # finishes early and its drain overlaps the
                            # remaining matmuls
                            engs = [nc.sync, nc.scalar, nc.gpsimd, nc.sync]
                            for ib in range(4):
                                group_mms(g, ib, xgrp, accs[ib])
                                drain_one(h, ib, accs, engs[ib])
                        else:
                            for ckl in range(GRP):
                                ck = g * GRP + ckl
                                for ib in range(4):
                                    nc.tensor.matmul(
                                        accs[ib][:],
                                        xgrp[:, ckl, ib * 128:(ib + 1) * 128],
                                        wh_aug[:, ck, 0:257],
                                        start=(ck == 0), stop=(ck == NCK - 1),
                                        skip_group_check=True)

                def drain(h, accs):
                    engs = [nc.sync, nc.scalar, nc.sync, nc.scalar]
                    for ib in range(4):
                        drain_one(h, ib, accs, engs[ib])

                accs0 = [psB.tile([128, 257], F32, name=f"acc{ib}",
                                  tag=f"acc{ib}") for ib in range(4)]
                accumulate(0, accs0, [], pre_masks)
                # build h=1's first score groups while the DVE is idle at
                # the tail of h=0
                prefetched = [emit_x(1, 0, "", 6, xbufs=5),
                              emit_x(1, 1, "", 6, xbufs=5),
                              emit_x(1, 2, "", 6, xbufs=5),
                              emit_x(1, 3, "", 6, xbufs=5)]
                psA_stack.close()   # free pwh/pw1 PSUM banks for h1 accs
                with tc.tile_pool(name="psC", bufs=1, space="PSUM") as psC:
                    accs1 = [psC.tile([128, 257], F32, name=f"acc1{ib}",
                                      tag=f"acc1{ib}") for ib in range(4)]
                    drain(0, accs0)
                    accumulate(1, accs1, prefetched, pre_masks,
                               drain_last=True)
    nc.compile()
    return nc


def _get_nc():
    if "nc" not in _CACHE:
        _CACHE["nc"] = _build_nc()
    return _CACHE["nc"]


def _prep_in_maps(nodes, edge_mat, W_w, W_b, a1_w, a1_b, a2_w, a2_b):
    f16 = np.float16
    nodes = np.asarray(nodes, dtype=np.float32)
    edge_mat = np.asarray(edge_mat, dtype=bool)
    W_w = np.asarray(W_w, dtype=np.float32)
    W_b = np.asarray(W_b, dtype=np.float32)
    a1_w = np.asarray(a1_w, dtype=np.float32)
    a1_b = np.asarray(a1_b, dtype=np.float32)
    a2_w = np.asarray(a2_w, dtype=np.float32)
    a2_b = np.asarray(a2_b, dtype=np.float32)

    nodesT = np.ascontiguousarray(nodes.T).astype(f16)          # [512, 8192]
    v1 = (W_w.T @ a1_w[0]).astype(f16)[:, None]                 # [512, 1]
    v2 = (W_w.T @ a2_w[0]).astype(f16)[:, None]
    wt_aug = np.concatenate([W_w.T.astype(f16), v2, v1], axis=1)  # [512, 258]
    c1v = float(W_b @ a1_w[0]) + float(a1_b[0])
    c2v = float(W_b @ a2_w[0]) + float(a2_b[0])
    c2qr = np.broadcast_to(
        np.array([ALPHA * c2v, c2v, c1v], np.float32)[None, :],
        (128, 3)).copy()
    wb_bc = np.ascontiguousarray(
        np.broadcast_to(W_b[None, :], (128, OUT_DIM))).astype(np.float32)
    # multiplicative {0,1} mask, transposed, fp8 (cast to fp16 during DMA)
    import ml_dtypes
    maskT = np.where(edge_mat, 1, 0).astype(ml_dtypes.float8_e4m3fn).T

    in_maps = []
    for c in range(N_CORES):
        sl = slice(c * ROWS, (c + 1) * ROWS)
        # permute node columns so this core's own block comes first; the
        # j-order of Wh/X/mask rows follows the same permutation (softmax
        # sums over j are permutation-invariant, output rows stay i-ordered)
        nT = np.concatenate(
            [nodesT[:, sl], nodesT[:, :sl.start], nodesT[:, sl.stop:]],
            axis=1)
        mcol = maskT[:, sl]
        mperm = np.concatenate(
            [mcol[sl.start:sl.stop], mcol[:sl.start], mcol[sl.stop:]], axis=0)
        in_maps.append({
            "nodesT": np.ascontiguousarray(nT),
            "maskm": np.ascontiguousarray(mperm),
            "wt_aug": wt_aug,
            "wb_bc": wb_bc,
            "c2qr": c2qr,
        })
    return in_maps


def _run(inputs, trace=False, trace_cores=None):
    from concourse.bass_utils import run_bass_kernel_spmd
    if trace:
        _ensure_ntff_hook()
    nc = _get_nc()
    in_maps = _prep_in_maps(**inputs)
    res = run_bass_kernel_spmd(nc, in_maps, list(range(N_CORES)),
                               trace=trace, trace_cores=trace_cores)
    out = np.concatenate([res.results[c]["out"] for c in range(N_CORES)],
                         axis=0)
    return out, res


def kernel(**inputs) -> np.ndarray:
    out, _ = _run(inputs, trace=False)
    return out
